# revision 18
# baseline (speedup 1.0000x reference)
"""Trainium2 Bass kernel for nn_Block_attention_guide (gnn_message_passing).

Computes, per sample (B=4096, J=17 joints, C=512 channels, K=4 gcn kernels,
H=256 uMLP hidden):
    xg = LN_joints(x); g1 = gcn(xg); g2 = gcn(g1); x2 = x + g2
    xn = LN_ch(x2);  u = uMLP(xn);   out = x2 + u
Pure data parallel over batch across 8 NeuronCores (512 samples/core).

Layout strategy per core: tiles of 7 samples = 119 tokens.  Activations
token-major (tokens on partitions) for the joint-aggregation matmuls; PE
transpose-matmuls produce channel-major operands (lhsT) for the convs/uMLP.
All matmuls run as float32r (full fp32 storage, reduced-precision multiply,
1 cycle/row at N>=256 vs 4 for fp32).  The joint aggregation is a matmul
against block-diagonal adjacency (per-tile block-diag of adj for k=0, fixed
kron(I7, spatial_adj_k) for k=1..3); LN gammas/betas, conv biases, s1/s2 and
both residuals are folded into matmuls / PSUM-copy scalars (no standalone
elementwise passes over the 2048-wide h).
"""

import sys

sys.path.insert(0, "/opt/trn_rl_repo")

from contextlib import ExitStack

import numpy as np

B, J, C, K, H = 4096, 17, 512, 4, 256
KC = K * C  # 2048
NCORES = 8
BS = B // NCORES  # samples per core
SPT = 7  # samples per tile (7*17 = 119 tokens <= 128 partitions)
GRP = 4  # tiles per uMLP group (batch the N dim to >=256 for float32r)
EPS = 1e-5

_cache: dict = {}


def _build(bs: int):
    """Emit + compile the per-core Bass module for a bs-sample shard."""
    import concourse.bacc as bacc
    import concourse.bass as bass
    import concourse.mybir as mybir
    from concourse.masks import make_identity
    from concourse.tile import TileContext

    dt = mybir.dt
    f32 = dt.float32
    AF = mybir.ActivationFunctionType
    OP = mybir.AluOpType
    AX = mybir.AxisListType

    ntiles = (bs + SPT - 1) // SPT

    nc = bacc.Bacc("TRN2", target_bir_lowering=False, debug=False, num_devices=NCORES)

    x_d = nc.dram_tensor("x", [bs, J, C], f32, kind="ExternalInput")
    adj_d = nc.dram_tensor("adj", [bs, 1, J, J], f32, kind="ExternalInput")
    sp_d = nc.dram_tensor("spatial_adj", [K - 1, J, J], f32, kind="ExternalInput")
    c1w_d = nc.dram_tensor("conv1_w", [KC, C], f32, kind="ExternalInput")
    c1b_d = nc.dram_tensor("conv1_b", [KC], f32, kind="ExternalInput")
    s1_d = nc.dram_tensor("s1", [1], f32, kind="ExternalInput")
    c2w_d = nc.dram_tensor("conv2_w", [KC, C], f32, kind="ExternalInput")
    c2b_d = nc.dram_tensor("conv2_b", [KC], f32, kind="ExternalInput")
    s2_d = nc.dram_tensor("s2", [1], f32, kind="ExternalInput")
    n1g_d = nc.dram_tensor("norm1_g", [J], f32, kind="ExternalInput")
    n1b_d = nc.dram_tensor("norm1_b", [J], f32, kind="ExternalInput")
    n2g_d = nc.dram_tensor("norm2_g", [C], f32, kind="ExternalInput")
    n2b_d = nc.dram_tensor("norm2_b", [C], f32, kind="ExternalInput")
    w1_d = nc.dram_tensor("w1", [C, H], f32, kind="ExternalInput")
    b1_d = nc.dram_tensor("b1", [H], f32, kind="ExternalInput")
    w2_d = nc.dram_tensor("w2", [H, H], f32, kind="ExternalInput")
    b2_d = nc.dram_tensor("b2", [H], f32, kind="ExternalInput")
    w3_d = nc.dram_tensor("w3", [H, C], f32, kind="ExternalInput")
    b3_d = nc.dram_tensor("b3", [C], f32, kind="ExternalInput")
    y_d = nc.dram_tensor("y", [bs, J, C], f32, kind="ExternalOutput")

    xf = x_d[:].rearrange("b j c -> (b j) c")
    yf = y_d[:].rearrange("b j c -> (b j) c")

    def r32(ap):
        return ap.bitcast(dt.float32r)

    def mm(out, lhsT, rhs, start, stop):
        nc.tensor.matmul(out, r32(lhsT), r32(rhs), start=start, stop=stop)

    def tmm(out, in_, ident, start, stop):
        # PE transpose-mode matmul: out = in_.T (identity rhs), fp32.
        nc.tensor.matmul(out, in_, ident, is_transpose=True, start=start, stop=stop)

    with TileContext(nc) as tc, ExitStack() as ctx:
        consts = ctx.enter_context(tc.tile_pool(name="consts", bufs=1))

        # ---------------- one-time constants ----------------
        ident = consts.tile([128, 128], f32)
        make_identity(nc, ident)

        osrc = consts.tile([128, 128], f32)
        nc.vector.memset(osrc, 1.0)
        zsrc = consts.tile([128, 128], f32)
        nc.vector.memset(zsrc, 0.0)
        ones_row = consts.tile([1, 128], f32)
        nc.vector.tensor_copy(out=r32(ones_row[:]), in_=osrc[:1, :])
        ones_col = consts.tile([128, 1], f32)
        nc.vector.tensor_copy(out=r32(ones_col[:]), in_=osrc[:, :1])
        eps_col = consts.tile([128, 1], f32)
        nc.vector.memset(eps_col, EPS)

        # per-token LN1 gamma column (gamma[j] tiled over samples) + beta row
        gJ_col = consts.tile([128, 1], f32)
        for s in range(SPT):
            nc.sync.dma_start(
                out=gJ_col[s * J : (s + 1) * J], in_=n1g_d[:][:, None]
            )
        bJ_row = consts.tile([1, 128], f32)
        nc.sync.dma_start(
            out=r32(bJ_row[:, : SPT * J].rearrange("o (s j) -> o s j", j=J)),
            in_=r32(n1b_d[:][None, None, :].to_broadcast((1, SPT, J))),
        )
        s1_col = consts.tile([128, 1], f32)
        nc.sync.dma_start(out=s1_col, in_=s1_d[:][:, None].to_broadcast((128, 1)))
        s2_col = consts.tile([128, 1], f32)
        nc.sync.dma_start(out=s2_col, in_=s2_d[:][:, None].to_broadcast((128, 1)))

        # LN2 gamma/beta as (128, C//128) columns, folded into w1/b1 below
        g2_col = consts.tile([128, C // 128], f32)
        nc.sync.dma_start(out=g2_col, in_=n2g_d[:].rearrange("(k p) -> p k", p=128))
        b2c_col = consts.tile([128, C // 128], f32)
        nc.sync.dma_start(
            out=r32(b2c_col[:]), in_=r32(n2b_d[:].rearrange("(k p) -> p k", p=128))
        )

        # uMLP weights, channel-major lhsT layout (contraction on partitions)
        w1_sb = consts.tile([128, C // 128, H], f32)
        nc.sync.dma_start(out=w1_sb, in_=w1_d[:].rearrange("(k p) h -> p k h", p=128))
        w1r = consts.tile([128, C // 128, H], f32)
        nc.vector.tensor_copy(out=r32(w1r[:]), in_=w1_sb[:])
        w2_sb = consts.tile([128, H // 128, H], f32)
        nc.sync.dma_start(
            out=r32(w2_sb[:]), in_=r32(w2_d[:].rearrange("(k p) h -> p k h", p=128))
        )
        w3_sb = consts.tile([128, H // 128, C], f32)
        nc.sync.dma_start(
            out=r32(w3_sb[:]), in_=r32(w3_d[:].rearrange("(k p) h -> p k h", p=128))
        )
        b1_col = consts.tile([128, H // 128], f32)
        nc.sync.dma_start(out=b1_col, in_=b1_d[:].rearrange("(k p) -> p k", p=128))
        b2_col = consts.tile([128, H // 128], f32)
        nc.sync.dma_start(out=b2_col, in_=b2_d[:].rearrange("(k p) -> p k", p=128))
        b3_col = consts.tile([128, C // 128], f32)
        nc.sync.dma_start(out=b3_col, in_=b3_d[:].rearrange("(k p) -> p k", p=128))

        # spatial adjacency -> kron(I_SPT, S_k) block-diagonal tiles
        TT = SPT * J  # 119
        bigS = []
        for k in range(K - 1):
            bs_k = consts.tile([TT, TT], f32, name=f"bigS{k}")
            nc.vector.tensor_copy(out=r32(bs_k[:]), in_=zsrc[:TT, :TT])
            for s in range(SPT):
                nc.sync.dma_start(
                    out=r32(bs_k[s * J : (s + 1) * J, s * J : (s + 1) * J]),
                    in_=r32(sp_d[k]),
                )
            bigS.append(bs_k)

        # per-tile block-diag adjacency, double buffered, zeroed once
        bigadj = []
        for i in range(2):
            ba = consts.tile([TT, TT], f32, name=f"bigadj{i}")
            nc.vector.tensor_copy(out=r32(ba[:]), in_=zsrc[:TT, :TT])
            bigadj.append(ba)

        identr = consts.tile([128, 128], f32)
        nc.vector.tensor_copy(out=r32(identr[:]), in_=ident[:])

        # conv weights: DMA o-major then PE-transpose to channel-major W^T
        W1T = consts.tile([128, C // 128, KC], f32)
        W2T = consts.tile([128, C // 128, KC], f32)
        # bias-fold constants (see module docstring):
        #   B1[v, o]  = betaJ[v] * rowsum(W1)[o] + conv1_b[o]
        #   B0s1      = s1 * B1[:, 0:C]          (k=0 block, per-tile adjacency)
        #   Cs1[w, c] = sum_{k>=1} (kron(I,S_k)^T @ B1_k)[w, c]   (fixed)
        B0s1 = consts.tile([TT, C], f32)
        Cs1 = consts.tile([TT, C], f32)
        B0s2 = consts.tile([TT, C], f32)
        Cs2 = consts.tile([TT, C], f32)

        with ExitStack() as setup_ctx:
            setup = setup_ctx.enter_context(tc.tile_pool(name="setup", bufs=1))
            spsum = setup_ctx.enter_context(
                tc.tile_pool(name="spsum", bufs=2, space="PSUM")
            )
            spsum1 = setup_ctx.enter_context(
                tc.tile_pool(name="spsum1", bufs=1, space="PSUM")
            )

            for wd, WT, cbd, B0s, Cs, s_col, with_beta in (
                (c1w_d, W1T, c1b_d, B0s1, Cs1, s1_col, True),
                (c2w_d, W2T, c2b_d, B0s2, Cs2, s2_col, False),
            ):
                Wo = setup.tile([128, KC // 128, C], f32, name="Wo")
                nc.sync.dma_start(
                    out=Wo, in_=wd[:].rearrange("(i p) c -> p i c", p=128)
                )
                for i in range(KC // 128):
                    for kc in range(C // 128):
                        pw = spsum.tile([128, 128], f32, tag="pw")
                        tmm(pw, Wo[:, i, kc * 128 : (kc + 1) * 128], ident, True, True)
                        if (i + kc) % 2 == 0:
                            nc.vector.tensor_copy(
                                out=r32(WT[:, kc, i * 128 : (i + 1) * 128]), in_=pw
                            )
                        else:
                            nc.scalar.copy(
                                out=r32(WT[:, kc, i * 128 : (i + 1) * 128]), in_=pw
                            )
                # rowsum(W)[o] as a (1, KC) row: ones^T @ W^T chunks
                rs_row = setup.tile([1, KC], f32, name="rs_row")
                cb_row = setup.tile([1, KC], f32, name="cb_row")
                nc.sync.dma_start(out=r32(cb_row[:]), in_=r32(cbd[:][None, :]))
                for n in range(K):
                    pr = spsum1.tile([1, C], f32, tag="pr")
                    for kc in range(C // 128):
                        mm(
                            pr,
                            ones_col,
                            WT[:, kc, n * C : (n + 1) * C],
                            start=(kc == 0),
                            stop=(kc == C // 128 - 1),
                        )
                    nc.vector.tensor_copy(
                        out=r32(rs_row[:, n * C : (n + 1) * C]), in_=pr
                    )
                # B[v, o] = betaJ[v]*rs[o] + conv_b[o]  (rank-1 + row via K=1 MMs)
                Bmat = setup.tile([TT, KC], f32, name="Bmat")
                for n in range(K):
                    pb = spsum.tile([TT, C], f32, tag="pb")
                    if with_beta:
                        mm(
                            pb,
                            bJ_row[:, :TT],
                            rs_row[:, n * C : (n + 1) * C],
                            start=True,
                            stop=False,
                        )
                        mm(
                            pb,
                            ones_row[:, :TT],
                            cb_row[:, n * C : (n + 1) * C],
                            start=False,
                            stop=True,
                        )
                    else:
                        mm(
                            pb,
                            ones_row[:, :TT],
                            cb_row[:, n * C : (n + 1) * C],
                            start=True,
                            stop=True,
                        )
                    nc.vector.tensor_copy(
                        out=r32(Bmat[:, n * C : (n + 1) * C]), in_=pb
                    )
                nc.vector.tensor_scalar(
                    out=r32(B0s[:]),
                    in0=Bmat[:, 0:C],
                    scalar1=s_col[:TT],
                    scalar2=None,
                    op0=OP.mult,
                )
                pc = spsum.tile([TT, C], f32, tag="pb")
                for k in range(1, K):
                    mm(
                        pc,
                        bigS[k - 1],
                        Bmat[:, k * C : (k + 1) * C],
                        start=(k == 1),
                        stop=(k == K - 1),
                    )
                nc.vector.tensor_copy(out=r32(Cs[:]), in_=pc)

            # fold LN2 gamma/beta into w1/b1:  w1' = gamma2[c]*w1;  b1' = b1 + beta2 @ w1
            for m in range(H // 128):
                pbc = spsum1.tile([1, 128], f32, tag="pr")
                for kc in range(C // 128):
                    mm(
                        pbc,
                        b2c_col[:, kc : kc + 1],
                        w1r[:, kc, m * 128 : (m + 1) * 128],
                        start=(kc == 0),
                        stop=(kc == C // 128 - 1),
                    )
                # transpose the (1,128) correction row to a (128,1) column and
                # add b1: via transpose-matmul of the psum row after copyback
                corr_row = setup.tile([1, 128], f32, name="corr_row")
                nc.vector.tensor_copy(out=corr_row, in_=pbc)
                pbt = spsum.tile([128, 128], f32, tag="pw")
                tmm(pbt[:, :1], corr_row, ident[:1, :1], True, True)
                nc.vector.tensor_tensor(
                    out=b1_col[:, m : m + 1],
                    in0=b1_col[:, m : m + 1],
                    in1=pbt[:, :1],
                    op=OP.add,
                )
            for kc in range(C // 128):
                nc.vector.tensor_scalar(
                    out=r32(w1r[:, kc, :]),
                    in0=w1_sb[:, kc, :],
                    scalar1=g2_col[:, kc : kc + 1],
                    scalar2=None,
                    op0=OP.mult,
                )

        # ---------------- main pipeline pools ----------------
        xpool = ctx.enter_context(tc.tile_pool(name="xpool", bufs=3))
        small = ctx.enter_context(tc.tile_pool(name="small", bufs=2))
        mid = ctx.enter_context(tc.tile_pool(name="mid", bufs=2))
        hpool = ctx.enter_context(tc.tile_pool(name="hpool", bufs=2))
        x2pool = ctx.enter_context(tc.tile_pool(name="x2pool", bufs=GRP + 2))
        grp_pool = ctx.enter_context(tc.tile_pool(name="grp", bufs=1))
        opool = ctx.enter_context(tc.tile_pool(name="opool", bufs=3))

        pt_pool = ctx.enter_context(tc.tile_pool(name="pt", bufs=2, space="PSUM"))
        ph_pool = ctx.enter_context(tc.tile_pool(name="ph", bufs=2, space="PSUM"))
        pg_pool = ctx.enter_context(tc.tile_pool(name="pg", bufs=3, space="PSUM"))
        pu_pool = ctx.enter_context(tc.tile_pool(name="pu", bufs=1, space="PSUM"))

        NCH = C // 128  # 4 channel chunks

        def transpose_tokmaj(src, T, tag, round_out=False):
            """(T, C) token-major SBUF -> (128, NCH, T) channel-major SBUF."""
            pt = pt_pool.tile([128, 512], f32, tag="pt")
            for c in range(NCH):
                tmm(
                    pt[:, c * T : (c + 1) * T],
                    src[:T, c * 128 : (c + 1) * 128],
                    ident[:T, :T],
                    start=(c == 0),
                    stop=(c == NCH - 1),
                )
            dst = mid.tile([128, NCH * 119], f32, name=tag, tag=tag)
            out_ap = dst[:, : NCH * T]
            nc.vector.tensor_copy(
                out=r32(out_ap) if round_out else out_ap, in_=pt[:, : NCH * T]
            )
            return dst[:, : NCH * T].rearrange("p (c t) -> p c t", t=T)

        def conv_gcn(xT, T, WT, B0s, Cs, scale_col, gamma_col):
            """One gcn layer: conv (16 MM) + h move + block-diag aggregation.

            xT: (128, NCH, T) channel-major input (lhsT chunks).
            Returns the (T, C) PSUM tile holding the aggregated output."""
            h_sb = hpool.tile([128, KC], f32, tag="h")
            pg = pg_pool.tile([128, 512], f32, tag="pg")
            ba = bigadj[_t % 2]
            for n in range(K):
                ph = ph_pool.tile([128, 512], f32, tag="ph")
                for kc in range(NCH):
                    mm(
                        ph[:T],
                        xT[:, kc, :],
                        WT[:, kc, n * C : (n + 1) * C],
                        start=(kc == 0),
                        stop=(kc == NCH - 1),
                    )
                # PSUM -> SBUF move with LN1-gamma / s-scale folded in
                hout = r32(h_sb[:T, n * C : (n + 1) * C])
                if n == 0:
                    if gamma_col is not None:
                        nc.vector.tensor_scalar(
                            out=hout, in0=ph[:T], scalar1=gamma_col[:T],
                            scalar2=scale_col[:T], op0=OP.mult, op1=OP.mult,
                        )
                    else:
                        nc.vector.tensor_scalar(
                            out=hout, in0=ph[:T], scalar1=scale_col[:T],
                            scalar2=None, op0=OP.mult,
                        )
                elif n in (1, 2):
                    if gamma_col is not None:
                        nc.scalar.activation(
                            out=hout, in_=ph[:T], func=AF.Copy, scale=gamma_col[:T]
                        )
                    else:
                        nc.scalar.copy(out=hout, in_=ph[:T])
                else:
                    if gamma_col is not None:
                        nc.vector.tensor_scalar(
                            out=hout, in0=ph[:T], scalar1=gamma_col[:T],
                            scalar2=None, op0=OP.mult,
                        )
                    else:
                        nc.vector.tensor_copy(out=hout, in_=ph[:T])
                lhs = ba if n == 0 else bigS[n - 1]
                mm(
                    pg[:T],
                    lhs[:T, :T],
                    h_sb[:T, n * C : (n + 1) * C],
                    start=(n == 0),
                    stop=False,
                )
            mm(pg[:T], ba[:T, :T], B0s[:T], start=False, stop=False)
            mm(pg[:T], identr[:T, :T], Cs[:T], start=False, stop=True)
            return pg

        ngrp = (ntiles + GRP - 1) // GRP
        inv17 = 1.0 / J
        invC = 1.0 / C

        for g in range(ngrp):
            g_tiles = range(g * GRP, min((g + 1) * GRP, ntiles))
            xnT_gr = grp_pool.tile([128, NCH, GRP * 119], f32, tag="xnT")
            x2_list = []
            off = 0
            offs = []
            Ts = []
            for _t in g_tiles:
                S = min(SPT, bs - _t * SPT)
                T = S * J
                t0 = _t * SPT * J  # token offset in the flat (bs*J, C) view

                x_t = xpool.tile([128, C], f32, tag="x")
                nc.sync.dma_start(out=x_t[:T], in_=xf[t0 : t0 + T])
                ba = bigadj[_t % 2]
                for s in range(S):
                    nc.sync.dma_start(
                        out=r32(ba[s * J : (s + 1) * J, s * J : (s + 1) * J]),
                        in_=r32(adj_d[_t * SPT + s, 0]),
                    )

                # ---- transpose x + LN1 (stats per (channel, sample)) ----
                xT = transpose_tokmaj(x_t, T, "xT")  # (128, NCH, T)
                G = NCH * S  # stat groups per partition
                xTg = xT.rearrange("p c (s j) -> p (c s) j", j=J)
                xsq = mid.tile([128, NCH * 119], f32, tag="xsq")
                nc.scalar.activation(
                    out=xsq[:, : NCH * T], in_=xT.rearrange("p c t -> p (c t)"),
                    func=AF.Square,
                )
                st = small.tile([128, 28, 6], f32, tag="st")
                nsum, sq, nmu, ex2, var, rstd = (st[:, :G, i] for i in range(6))
                nc.vector.tensor_reduce(
                    out=nsum, in_=xTg, axis=AX.X, op=OP.add, negate=True
                )
                nc.vector.tensor_reduce(
                    out=sq,
                    in_=xsq[:, : NCH * T].rearrange("p (g j) -> p g j", j=J),
                    axis=AX.X,
                    op=OP.add,
                )
                nc.vector.tensor_scalar(
                    out=nmu, in0=nsum, scalar1=inv17, scalar2=None, op0=OP.mult
                )
                nc.vector.tensor_scalar(
                    out=ex2, in0=sq, scalar1=inv17, scalar2=None, op0=OP.mult
                )
                nc.vector.tensor_tensor(out=var, in0=nmu, in1=nmu, op=OP.mult)
                nc.vector.tensor_tensor(out=var, in0=ex2, in1=var, op=OP.subtract)
                nc.scalar.activation(
                    out=rstd, in_=var, func=AF.Sqrt, bias=eps_col, scale=1.0
                )
                nc.vector.reciprocal(out=rstd, in_=rstd)
                # apply (x - mu) * rstd on gpsimd (gamma/beta folded elsewhere)
                xc = mid.tile([128, NCH * 119], f32, tag="xc")
                xcg = xc[:, : NCH * T].rearrange("p (g j) -> p g j", j=J)
                nc.gpsimd.tensor_tensor(
                    out=xcg,
                    in0=xTg,
                    in1=nmu[:, :, None].to_broadcast((128, G, J)),
                    op=OP.add,
                )
                xgT = mid.tile([128, NCH * 119], f32, tag="xgT")
                xgTg = xgT[:, : NCH * T].rearrange("p (g j) -> p g j", j=J)
                nc.vector.tensor_tensor(
                    out=r32(xgTg),
                    in0=xcg,
                    in1=rstd[:, :, None].to_broadcast((128, G, J)),
                    op=OP.mult,
                )

                # ---- gcn layer 1 ----
                pg1 = conv_gcn(
                    xgT[:, : NCH * T].rearrange("p (c t) -> p c t", t=T),
                    T, W1T, B0s1, Cs1, s1_col, gJ_col,
                )
                g1_sb = mid.tile([128, C], f32, tag="g1")
                nc.scalar.copy(out=g1_sb[:T], in_=pg1[:T])

                # ---- gcn layer 2 (+ residual x) ----
                g1T = transpose_tokmaj(g1_sb, T, "g1T", round_out=True)
                pg2 = conv_gcn(g1T, T, W2T, B0s2, Cs2, s2_col, None)
                # x2 = x + g2 (DVE add straight out of PSUM, f32r-rounded)
                x2_sb = x2pool.tile([128, C], f32, tag="x2")
                nc.vector.tensor_tensor(
                    out=r32(x2_sb[:T]), in0=pg2[:T], in1=x_t[:T], op=OP.add
                )

                # ---- LN2 (token-major; gamma2/beta2 folded into w1/b1) ----
                st2 = small.tile([128, 8], f32, tag="st2")
                nc.vector.tensor_reduce(
                    out=st2[:T, 0:1], in_=x2_sb[:T], axis=AX.X, op=OP.add, negate=True
                )
                scr2 = mid.tile([128, C], f32, tag="scr2")
                nc.scalar.activation(
                    out=scr2[:T],
                    in_=x2_sb[:T],
                    func=AF.Square,
                    accum_out=st2[:T, 1:2],
                )
                nc.vector.tensor_scalar(
                    out=st2[:T, 2:3], in0=st2[:T, 0:1], scalar1=invC,
                    scalar2=None, op0=OP.mult,
                )
                nc.vector.tensor_scalar(
                    out=st2[:T, 3:4], in0=st2[:T, 1:2], scalar1=invC,
                    scalar2=None, op0=OP.mult,
                )
                nc.vector.tensor_tensor(
                    out=st2[:T, 4:5], in0=st2[:T, 2:3], in1=st2[:T, 2:3], op=OP.mult
                )
                nc.vector.tensor_tensor(
                    out=st2[:T, 4:5], in0=st2[:T, 3:4], in1=st2[:T, 4:5],
                    op=OP.subtract,
                )
                nc.scalar.activation(
                    out=st2[:T, 5:6], in_=st2[:T, 4:5], func=AF.Sqrt,
                    bias=eps_col[:T], scale=1.0,
                )
                nc.vector.reciprocal(out=st2[:T, 5:6], in_=st2[:T, 5:6])
                xn_sb = mid.tile([128, C], f32, tag="xn")
                nc.vector.tensor_scalar(
                    out=xn_sb[:T],
                    in0=x2_sb[:T],
                    scalar1=st2[:T, 2:3],
                    scalar2=st2[:T, 5:6],
                    op0=OP.add,
                    op1=OP.mult,
                )

                # ---- transpose xn into the group buffer ----
                pt = pt_pool.tile([128, 512], f32, tag="pt")
                for c in range(NCH):
                    tmm(
                        pt[:, c * T : (c + 1) * T],
                        xn_sb[:T, c * 128 : (c + 1) * 128],
                        ident[:T, :T],
                        start=(c == 0),
                        stop=(c == NCH - 1),
                    )
                nc.vector.tensor_copy(
                    out=r32(xnT_gr[:, :, off : off + T]),
                    in_=pt[:, : NCH * T].rearrange("p (c t) -> p c t", t=T),
                )
                x2_list.append(x2_sb)
                offs.append(off)
                Ts.append(T)
                off += T

            # ---- uMLP over the whole group (channel-major, N=off tokens) ----
            W = off
            h1_sb = grp_pool.tile([128, H // 128, GRP * 119], f32, tag="h1")
            for m in range(H // 128):
                pu = pu_pool.tile([128, GRP * 119], f32, tag="pu")
                for kc in range(NCH):
                    mm(
                        pu[:, :W],
                        w1r[:, kc, m * 128 : (m + 1) * 128],
                        xnT_gr[:, kc, :W],
                        start=(kc == 0),
                        stop=(kc == NCH - 1),
                    )
                nc.scalar.activation(
                    out=r32(h1_sb[:, m, :W]), in_=pu[:, :W], func=AF.Gelu,
                    bias=b1_col[:, m : m + 1], scale=1.0,
                )
            h2_sb = grp_pool.tile([128, H // 128, GRP * 119], f32, tag="h2")
            for m in range(H // 128):
                pu = pu_pool.tile([128, GRP * 119], f32, tag="pu")
                for kc in range(H // 128):
                    mm(
                        pu[:, :W],
                        w2_sb[:, kc, m * 128 : (m + 1) * 128],
                        h1_sb[:, kc, :W],
                        start=(kc == 0),
                        stop=(kc == H // 128 - 1),
                    )
                nc.scalar.activation(
                    out=r32(h2_sb[:, m, :W]), in_=pu[:, :W], func=AF.Gelu,
                    bias=b2_col[:, m : m + 1], scale=1.0,
                )
            h3_sb = grp_pool.tile([128, NCH, GRP * 119], f32, tag="h3")
            for m in range(NCH):
                pu = pu_pool.tile([128, GRP * 119], f32, tag="pu")
                for kc in range(H // 128):
                    # (h2g + h1) @ w3: residual folded as a second accumulate
                    mm(
                        pu[:, :W],
                        w3_sb[:, kc, m * 128 : (m + 1) * 128],
                        h2_sb[:, kc, :W],
                        start=(kc == 0),
                        stop=False,
                    )
                    mm(
                        pu[:, :W],
                        w3_sb[:, kc, m * 128 : (m + 1) * 128],
                        h1_sb[:, kc, :W],
                        start=False,
                        stop=(kc == H // 128 - 1),
                    )
                nc.scalar.activation(
                    out=h3_sb[:, m, :W], in_=pu[:, :W], func=AF.Gelu,
                    bias=b3_col[:, m : m + 1], scale=1.0,
                )

            # ---- per tile: transpose back to token-major, + x2, store ----
            for i, _t in enumerate(g_tiles):
                T = Ts[i]
                t0 = _t * SPT * J
                po = pg_pool.tile([128, 512], f32, tag="pg")
                for c in range(NCH):
                    tmm(
                        po[:T, c * 128 : (c + 1) * 128],
                        h3_sb[:, c, offs[i] : offs[i] + T],
                        ident,
                        start=(c == 0),
                        stop=False,
                    )
                mm(po[:T], identr[:T, :T], x2_list[i][:T], start=False, stop=True)
                out_sb = opool.tile([128, C], f32, tag="out")
                nc.scalar.copy(out=out_sb[:T], in_=po[:T])
                nc.sync.dma_start(out=yf[t0 : t0 + T], in_=out_sb[:T])

    nc.compile()
    return nc


def _get(bs: int):
    if bs not in _cache:
        _cache[bs] = _build(bs)
    return _cache[bs]


def kernel(**inputs) -> np.ndarray:
    from concourse.bass_utils import run_bass_kernel_spmd

    x = np.ascontiguousarray(np.asarray(inputs["x"], dtype=np.float32))
    adj = np.ascontiguousarray(np.asarray(inputs["adj"], dtype=np.float32))
    rest = {
        k: np.ascontiguousarray(np.asarray(v, dtype=np.float32))
        for k, v in inputs.items()
        if k not in ("x", "adj")
    }
    bs = x.shape[0] // NCORES
    nc = _get(bs)
    in_maps = []
    for c in range(NCORES):
        m = {"x": x[c * bs : (c + 1) * bs], "adj": adj[c * bs : (c + 1) * bs]}
        m.update(rest)
        in_maps.append(m)
    res = run_bass_kernel_spmd(nc, in_maps, core_ids=list(range(NCORES)))
    return np.concatenate([res.results[c]["y"] for c in range(NCORES)], axis=0)


# revision 34
# speedup vs baseline: 1.5279x; 1.5279x over previous
"""Trainium2 Bass kernel for nn_Block_attention_guide (gnn_message_passing).

Computes, per sample (B=4096, J=17 joints, C=512 channels, K=4 gcn kernels,
H=256 uMLP hidden):
    xg = LN_joints(x); g1 = gcn(xg); g2 = gcn(g1); x2 = x + g2
    xn = LN_ch(x2);  u = uMLP(xn);   out = x2 + u
Pure data parallel over batch across 8 NeuronCores (512 samples/core).

Layout strategy per core: tiles of 7 samples = 119 tokens.  Activations
token-major (tokens on partitions) for the joint-aggregation matmuls; PE
transpose-matmuls produce channel-major operands (lhsT) for the convs/uMLP.
All matmuls run as float32r (full fp32 storage, reduced-precision multiply,
1 cycle/row at N>=256 vs 4 for fp32).  The joint aggregation is a matmul
against block-diagonal adjacency (per-tile block-diag of adj for k=0, fixed
kron(I7, spatial_adj_k) for k=1..3); LN gammas/betas, conv biases, s1/s2 and
both residuals are folded into matmuls / PSUM-copy scalars (no standalone
elementwise passes over the 2048-wide h).
"""

import sys

sys.path.insert(0, "/opt/trn_rl_repo")

from contextlib import ExitStack

import numpy as np

B, J, C, K, H = 4096, 17, 512, 4, 256
KC = K * C  # 2048
NCORES = 8
BS = B // NCORES  # samples per core
SPT = 7  # samples per tile (7*17 = 119 tokens <= 128 partitions)
GRP = 4  # tiles per uMLP group (batch the N dim to >=256 for float32r)
EPS = 1e-5

_cache: dict = {}


def _build(bs: int):
    """Emit + compile the per-core Bass module for a bs-sample shard."""
    import concourse.bacc as bacc
    import concourse.bass as bass
    import concourse.mybir as mybir
    from concourse.masks import make_identity
    from concourse.tile import TileContext

    dt = mybir.dt
    f32 = dt.float32
    AF = mybir.ActivationFunctionType
    OP = mybir.AluOpType
    AX = mybir.AxisListType

    ntiles = (bs + SPT - 1) // SPT

    nc = bacc.Bacc("TRN2", target_bir_lowering=False, debug=False, num_devices=NCORES)

    x_d = nc.dram_tensor("x", [bs, J, C], f32, kind="ExternalInput")
    adj_d = nc.dram_tensor("adj", [bs, 1, J, J], f32, kind="ExternalInput")
    sp_d = nc.dram_tensor("spatial_adj", [K - 1, J, J], f32, kind="ExternalInput")
    c1w_d = nc.dram_tensor("conv1_w", [KC, C], f32, kind="ExternalInput")
    c1b_d = nc.dram_tensor("conv1_b", [KC], f32, kind="ExternalInput")
    s1_d = nc.dram_tensor("s1", [1], f32, kind="ExternalInput")
    c2w_d = nc.dram_tensor("conv2_w", [KC, C], f32, kind="ExternalInput")
    c2b_d = nc.dram_tensor("conv2_b", [KC], f32, kind="ExternalInput")
    s2_d = nc.dram_tensor("s2", [1], f32, kind="ExternalInput")
    n1g_d = nc.dram_tensor("norm1_g", [J], f32, kind="ExternalInput")
    n1b_d = nc.dram_tensor("norm1_b", [J], f32, kind="ExternalInput")
    n2g_d = nc.dram_tensor("norm2_g", [C], f32, kind="ExternalInput")
    n2b_d = nc.dram_tensor("norm2_b", [C], f32, kind="ExternalInput")
    w1_d = nc.dram_tensor("w1", [C, H], f32, kind="ExternalInput")
    b1_d = nc.dram_tensor("b1", [H], f32, kind="ExternalInput")
    w2_d = nc.dram_tensor("w2", [H, H], f32, kind="ExternalInput")
    b2_d = nc.dram_tensor("b2", [H], f32, kind="ExternalInput")
    w3_d = nc.dram_tensor("w3", [H, C], f32, kind="ExternalInput")
    b3_d = nc.dram_tensor("b3", [C], f32, kind="ExternalInput")
    y_d = nc.dram_tensor("y", [bs, J, C], f32, kind="ExternalOutput")

    xf = x_d[:].rearrange("b j c -> (b j) c")
    yf = y_d[:].rearrange("b j c -> (b j) c")

    def r32(ap):
        return ap.bitcast(dt.float32r)

    def mm(out, lhsT, rhs, start, stop):
        nc.tensor.matmul(out, r32(lhsT), r32(rhs), start=start, stop=stop)

    def tmm(out, in_, ident, start, stop):
        # PE transpose-mode matmul: out = in_.T (identity rhs), fp32.
        nc.tensor.matmul(out, in_, ident, is_transpose=True, start=start, stop=stop)

    with TileContext(nc) as tc, ExitStack() as ctx:
        consts = ctx.enter_context(tc.tile_pool(name="consts", bufs=1))

        # ---------------- one-time constants ----------------
        ident = consts.tile([128, 128], f32)
        make_identity(nc, ident)

        osrc = consts.tile([128, 128], f32)
        nc.vector.memset(osrc, 1.0)
        zsrc = consts.tile([128, 128], f32)
        nc.vector.memset(zsrc, 0.0)
        ones_row = consts.tile([1, 128], f32)
        nc.vector.tensor_copy(out=r32(ones_row[:]), in_=osrc[:1, :])
        ones_col = consts.tile([128, 1], f32)
        nc.vector.tensor_copy(out=r32(ones_col[:]), in_=osrc[:, :1])
        eps_col = consts.tile([128, 1], f32)
        nc.vector.memset(eps_col, EPS)

        # per-token LN1 gamma column (gamma[j] tiled over samples) + beta row
        gJ_col = consts.tile([128, 1], f32)
        for s in range(SPT):
            nc.sync.dma_start(
                out=gJ_col[s * J : (s + 1) * J], in_=n1g_d[:][:, None]
            )
        bJ_row = consts.tile([1, 128], f32)
        nc.sync.dma_start(
            out=r32(bJ_row[:, : SPT * J].rearrange("o (s j) -> o s j", j=J)),
            in_=r32(n1b_d[:][None, None, :].to_broadcast((1, SPT, J))),
        )
        s1_col = consts.tile([128, 1], f32)
        nc.sync.dma_start(out=s1_col, in_=s1_d[:][:, None].to_broadcast((128, 1)))
        s2_col = consts.tile([128, 1], f32)
        nc.sync.dma_start(out=s2_col, in_=s2_d[:][:, None].to_broadcast((128, 1)))

        # LN2 gamma/beta as (128, C//128) columns, folded into w1/b1 below
        g2_col = consts.tile([128, C // 128], f32)
        nc.sync.dma_start(out=g2_col, in_=n2g_d[:].rearrange("(k p) -> p k", p=128))
        b2c_col = consts.tile([128, C // 128], f32)
        nc.sync.dma_start(
            out=r32(b2c_col[:]), in_=r32(n2b_d[:].rearrange("(k p) -> p k", p=128))
        )

        # uMLP weights, channel-major lhsT layout (contraction on partitions)
        w1_sb = consts.tile([128, C // 128, H], f32)
        nc.sync.dma_start(out=w1_sb, in_=w1_d[:].rearrange("(k p) h -> p k h", p=128))
        w1r = consts.tile([128, C // 128, H], f32)
        nc.vector.tensor_copy(out=r32(w1r[:]), in_=w1_sb[:])
        w2_sb = consts.tile([128, H // 128, H], f32)
        nc.sync.dma_start(
            out=r32(w2_sb[:]), in_=r32(w2_d[:].rearrange("(k p) h -> p k h", p=128))
        )
        w3_sb = consts.tile([128, H // 128, C], f32)
        nc.sync.dma_start(
            out=r32(w3_sb[:]), in_=r32(w3_d[:].rearrange("(k p) h -> p k h", p=128))
        )
        b1_col = consts.tile([128, H // 128], f32)
        nc.sync.dma_start(out=b1_col, in_=b1_d[:].rearrange("(k p) -> p k", p=128))
        b2_col = consts.tile([128, H // 128], f32)
        nc.sync.dma_start(out=b2_col, in_=b2_d[:].rearrange("(k p) -> p k", p=128))
        b3_col = consts.tile([128, C // 128], f32)
        nc.sync.dma_start(out=b3_col, in_=b3_d[:].rearrange("(k p) -> p k", p=128))

        # spatial adjacency -> kron(I_SPT, S_k) block-diagonal tiles
        TT = SPT * J  # 119
        bigS = []
        for k in range(K - 1):
            bs_k = consts.tile([TT, TT], f32, name=f"bigS{k}")
            nc.vector.tensor_copy(out=r32(bs_k[:]), in_=zsrc[:TT, :TT])
            for s in range(SPT):
                nc.sync.dma_start(
                    out=r32(bs_k[s * J : (s + 1) * J, s * J : (s + 1) * J]),
                    in_=r32(sp_d[k]),
                )
            bigS.append(bs_k)

        # per-tile block-diag adjacency, double buffered (built on PE below)
        bigadj = []
        for i in range(2):
            ba = consts.tile([TT, TT], f32, name=f"bigadj{i}")
            nc.vector.tensor_copy(out=r32(ba[:]), in_=zsrc[:TT, :TT])
            bigadj.append(ba)
        # M_s = diag(indicator of sample-s rows): BigAdj0 col-block s is
        # M_s^T @ adj_stage — keeps the per-tile adjacency load to ONE DMA
        # (HWDGE fixed overhead is ~0.65us per DMA on a shared device).
        msel = []
        for s_i in range(SPT):
            m_s = consts.tile([TT, TT], f32, name=f"msel{s_i}")
            nc.vector.tensor_copy(out=r32(m_s[:]), in_=zsrc[:TT, :TT])
            nc.sync.dma_start(
                out=r32(m_s[s_i * J : (s_i + 1) * J, s_i * J : (s_i + 1) * J]),
                in_=r32(ident[:J, :J]),
            )
            msel.append(m_s)

        identr = consts.tile([128, 128], f32)
        nc.vector.tensor_copy(out=r32(identr[:]), in_=ident[:])

        # conv weights: DMA o-major then PE-transpose to channel-major W^T
        W1T = consts.tile([128, C // 128, KC], f32)
        W2T = consts.tile([128, C // 128, KC], f32)
        # bias-fold constants (see module docstring):
        #   B1[v, o]  = betaJ[v] * rowsum(W1)[o] + conv1_b[o]
        #   B0s1      = s1 * B1[:, 0:C]          (k=0 block, per-tile adjacency)
        #   Cs1[w, c] = sum_{k>=1} (kron(I,S_k)^T @ B1_k)[w, c]   (fixed)
        B0s1 = consts.tile([TT, C], f32)
        Cs1 = consts.tile([TT, C], f32)
        B0s2 = consts.tile([TT, C], f32)
        Cs2 = consts.tile([TT, C], f32)

        with ExitStack() as setup_ctx:
            setup = setup_ctx.enter_context(tc.tile_pool(name="setup", bufs=1))
            spsum = setup_ctx.enter_context(
                tc.tile_pool(name="spsum", bufs=2, space="PSUM")
            )
            spsum1 = setup_ctx.enter_context(
                tc.tile_pool(name="spsum1", bufs=1, space="PSUM")
            )

            for wd, WT, cbd, B0s, Cs, s_col, with_beta in (
                (c1w_d, W1T, c1b_d, B0s1, Cs1, s1_col, True),
                (c2w_d, W2T, c2b_d, B0s2, Cs2, s2_col, False),
            ):
                Wo = setup.tile([128, KC // 128, C], f32, name="Wo")
                nc.sync.dma_start(
                    out=Wo, in_=wd[:].rearrange("(i p) c -> p i c", p=128)
                )
                for i in range(KC // 128):
                    for kc in range(C // 128):
                        pw = spsum.tile([128, 128], f32, tag="pw")
                        tmm(pw, Wo[:, i, kc * 128 : (kc + 1) * 128], ident, True, True)
                        if (i + kc) % 2 == 0:
                            nc.vector.tensor_copy(
                                out=r32(WT[:, kc, i * 128 : (i + 1) * 128]), in_=pw
                            )
                        else:
                            nc.scalar.copy(
                                out=r32(WT[:, kc, i * 128 : (i + 1) * 128]), in_=pw
                            )
                # rowsum(W)[o] as a (1, KC) row: ones^T @ W^T chunks
                rs_row = setup.tile([1, KC], f32, name="rs_row")
                cb_row = setup.tile([1, KC], f32, name="cb_row")
                nc.sync.dma_start(out=r32(cb_row[:]), in_=r32(cbd[:][None, :]))
                for n in range(K):
                    pr = spsum1.tile([1, C], f32, tag="pr")
                    for kc in range(C // 128):
                        mm(
                            pr,
                            ones_col,
                            WT[:, kc, n * C : (n + 1) * C],
                            start=(kc == 0),
                            stop=(kc == C // 128 - 1),
                        )
                    nc.vector.tensor_copy(
                        out=r32(rs_row[:, n * C : (n + 1) * C]), in_=pr
                    )
                # B[v, o] = betaJ[v]*rs[o] + conv_b[o]  (rank-1 + row via K=1 MMs)
                Bmat = setup.tile([TT, KC], f32, name="Bmat")
                for n in range(K):
                    pb = spsum.tile([TT, C], f32, tag="pb")
                    if with_beta:
                        mm(
                            pb,
                            bJ_row[:, :TT],
                            rs_row[:, n * C : (n + 1) * C],
                            start=True,
                            stop=False,
                        )
                        mm(
                            pb,
                            ones_row[:, :TT],
                            cb_row[:, n * C : (n + 1) * C],
                            start=False,
                            stop=True,
                        )
                    else:
                        mm(
                            pb,
                            ones_row[:, :TT],
                            cb_row[:, n * C : (n + 1) * C],
                            start=True,
                            stop=True,
                        )
                    nc.vector.tensor_copy(
                        out=r32(Bmat[:, n * C : (n + 1) * C]), in_=pb
                    )
                nc.vector.tensor_scalar(
                    out=r32(B0s[:]),
                    in0=Bmat[:, 0:C],
                    scalar1=s_col[:TT],
                    scalar2=None,
                    op0=OP.mult,
                )
                pc = spsum.tile([TT, C], f32, tag="pb")
                for k in range(1, K):
                    mm(
                        pc,
                        bigS[k - 1],
                        Bmat[:, k * C : (k + 1) * C],
                        start=(k == 1),
                        stop=(k == K - 1),
                    )
                nc.vector.tensor_copy(out=r32(Cs[:]), in_=pc)

            # fold LN2 gamma/beta into w1/b1:  w1' = gamma2[c]*w1;  b1' = b1 + beta2 @ w1
            for m in range(H // 128):
                pbc = spsum1.tile([1, 128], f32, tag="pr")
                for kc in range(C // 128):
                    mm(
                        pbc,
                        b2c_col[:, kc : kc + 1],
                        w1r[:, kc, m * 128 : (m + 1) * 128],
                        start=(kc == 0),
                        stop=(kc == C // 128 - 1),
                    )
                # transpose the (1,128) correction row to a (128,1) column and
                # add b1: via transpose-matmul of the psum row after copyback
                corr_row = setup.tile([1, 128], f32, name="corr_row")
                nc.vector.tensor_copy(out=corr_row, in_=pbc)
                pbt = spsum.tile([128, 128], f32, tag="pw")
                tmm(pbt[:, :1], corr_row, ident[:1, :1], True, True)
                nc.vector.tensor_tensor(
                    out=b1_col[:, m : m + 1],
                    in0=b1_col[:, m : m + 1],
                    in1=pbt[:, :1],
                    op=OP.add,
                )
            for kc in range(C // 128):
                nc.vector.tensor_scalar(
                    out=r32(w1r[:, kc, :]),
                    in0=w1_sb[:, kc, :],
                    scalar1=g2_col[:, kc : kc + 1],
                    scalar2=None,
                    op0=OP.mult,
                )

        # ---------------- main pipeline pools ----------------
        xpool = ctx.enter_context(tc.tile_pool(name="xpool", bufs=3))
        small = ctx.enter_context(tc.tile_pool(name="small", bufs=3))
        mid = ctx.enter_context(tc.tile_pool(name="mid", bufs=3))
        hpool = ctx.enter_context(tc.tile_pool(name="hpool", bufs=2))
        x2pool = ctx.enter_context(tc.tile_pool(name="x2pool", bufs=GRP + 1))
        grp_pool = ctx.enter_context(tc.tile_pool(name="grp", bufs=1))
        grp2_pool = ctx.enter_context(tc.tile_pool(name="grp2", bufs=2))
        scr_pool = ctx.enter_context(tc.tile_pool(name="scr", bufs=2))
        opool = ctx.enter_context(tc.tile_pool(name="opool", bufs=2))

        pt_pool = ctx.enter_context(tc.tile_pool(name="pt", bufs=2, space="PSUM"))
        ph_pool = ctx.enter_context(tc.tile_pool(name="ph", bufs=2, space="PSUM"))
        pg_pool = ctx.enter_context(tc.tile_pool(name="pg", bufs=2, space="PSUM"))
        pu_pool = ctx.enter_context(tc.tile_pool(name="pu", bufs=2, space="PSUM"))

        NCH = C // 128  # 4 channel chunks

        def transpose_tokmaj(src, T, tag, round_out=False):
            """(T, C) token-major SBUF -> (128, NCH, T) channel-major SBUF."""
            pt = pt_pool.tile([128, 512], f32, tag="pt")
            for c in range(NCH):
                tmm(
                    pt[:, c * T : (c + 1) * T],
                    src[:T, c * 128 : (c + 1) * 128],
                    ident[:T, :T],
                    start=(c == 0),
                    stop=(c == NCH - 1),
                )
            dst = mid.tile([128, NCH * 119], f32, name=tag, tag=tag)
            out_ap = dst[:, : NCH * T]
            nc.vector.tensor_copy(
                out=r32(out_ap) if round_out else out_ap, in_=pt[:, : NCH * T]
            )
            return dst[:, : NCH * T].rearrange("p (c t) -> p c t", t=T)

        def conv_gcn(xT, T, WT, B0s, Cs, scale_col, gamma_col, ba):
            """One gcn layer: conv (16 MM) + h move + block-diag aggregation.

            xT: (128, NCH, T) channel-major input (lhsT chunks).
            Returns the (T, C) PSUM tile holding the aggregated output."""
            h_sb = hpool.tile([128, KC], f32, tag="h")
            pg = pg_pool.tile([128, 512], f32, tag="pg")
            for n in range(K):
                ph = ph_pool.tile([128, 512], f32, tag="ph")
                for kc in range(NCH):
                    mm(
                        ph[:T],
                        xT[:, kc, :],
                        WT[:, kc, n * C : (n + 1) * C],
                        start=(kc == 0),
                        stop=(kc == NCH - 1),
                    )
                # PSUM -> SBUF move with LN1-gamma / s-scale folded in
                hout = r32(h_sb[:T, n * C : (n + 1) * C])
                if n == 0:
                    if gamma_col is not None:
                        nc.vector.tensor_scalar(
                            out=hout, in0=ph[:T], scalar1=gamma_col[:T],
                            scalar2=scale_col[:T], op0=OP.mult, op1=OP.mult,
                        )
                    else:
                        nc.vector.tensor_scalar(
                            out=hout, in0=ph[:T], scalar1=scale_col[:T],
                            scalar2=None, op0=OP.mult,
                        )
                elif n in (1, 2):
                    if gamma_col is not None:
                        nc.scalar.activation(
                            out=hout, in_=ph[:T], func=AF.Copy, scale=gamma_col[:T]
                        )
                    else:
                        nc.scalar.copy(out=hout, in_=ph[:T])
                else:
                    if gamma_col is not None:
                        nc.vector.tensor_scalar(
                            out=hout, in0=ph[:T], scalar1=gamma_col[:T],
                            scalar2=None, op0=OP.mult,
                        )
                    else:
                        nc.vector.tensor_copy(out=hout, in_=ph[:T])
                lhs = ba if n == 0 else bigS[n - 1]
                mm(
                    pg[:T],
                    lhs[:T, :T],
                    h_sb[:T, n * C : (n + 1) * C],
                    start=(n == 0),
                    stop=False,
                )
            mm(pg[:T], ba[:T, :T], B0s[:T], start=False, stop=False)
            mm(pg[:T], identr[:T, :T], Cs[:T], start=False, stop=True)
            return pg

        ngrp = (ntiles + GRP - 1) // GRP
        inv17 = 1.0 / J
        invC = 1.0 / C

        def front(_t):
            """Load + adjacency scatter + x transpose + LN1 -> xgT.

            Emitted one tile ahead of heavy() so its long DVE/ACT/Pool
            dependency chain overlaps the previous tile's conv matmuls."""
            S = min(SPT, bs - _t * SPT)
            T = S * J
            t0 = _t * SPT * J  # token offset in the flat (bs*J, C) view

            x_t = xpool.tile([128, C], f32, tag="x")
            nc.sync.dma_start(out=x_t[:T], in_=xf[t0 : t0 + T])
            ba = bigadj[_t % 2]
            # fp32r matmul requires an even moving free dim: stage the 17-wide
            # adjacency at pitch 18 and compact on the PSUM->SBUF copy.
            adj_t = small.tile([128, J + 1], f32, tag="adj")
            nc.vector.tensor_copy(out=r32(adj_t[:, J : J + 1]), in_=zsrc[:, :1])
            nc.sync.dma_start(
                out=r32(adj_t[:T, :J]),
                in_=r32(
                    adj_d[_t * SPT : _t * SPT + S, 0].rearrange("s v w -> (s v) w")
                ),
            )
            pba = pu_pool.tile([128, 476], f32, tag="pu")
            for s in range(S):
                mm(
                    pba[:T, s * (J + 1) : (s + 1) * (J + 1)],
                    msel[s][:T, :T],
                    adj_t[:T],
                    start=(s == 0),
                    stop=(s == S - 1),
                )
            nc.vector.tensor_copy(
                out=r32(ba[:T, :T].rearrange("p (s w) -> p s w", w=J)),
                in_=pba[:T, : S * (J + 1)].rearrange(
                    "p (s w) -> p s w", w=J + 1
                )[:, :, :J],
            )

            # ---- transpose x + LN1 (stats per (channel, sample)) ----
            xT = transpose_tokmaj(x_t, T, "xT")  # (128, NCH, T)
            G = NCH * S  # stat groups per partition
            xTg = xT.rearrange("p c (s j) -> p (c s) j", j=J)
            xsq = scr_pool.tile([128, NCH * 119], f32, tag="xsq")
            nc.scalar.activation(
                out=xsq[:, : NCH * T], in_=xT.rearrange("p c t -> p (c t)"),
                func=AF.Square,
            )
            st = small.tile([128, 28, 6], f32, tag="st")
            nsum, sq, nmu, ex2, var, rstd = (st[:, :G, i] for i in range(6))
            nc.vector.tensor_reduce(
                out=nsum, in_=xTg, axis=AX.X, op=OP.add, negate=True
            )
            nc.vector.tensor_reduce(
                out=sq,
                in_=xsq[:, : NCH * T].rearrange("p (g j) -> p g j", j=J),
                axis=AX.X,
                op=OP.add,
            )
            nc.vector.tensor_scalar(
                out=nmu, in0=nsum, scalar1=inv17, scalar2=None, op0=OP.mult
            )
            nc.vector.tensor_scalar(
                out=ex2, in0=sq, scalar1=inv17, scalar2=None, op0=OP.mult
            )
            nc.vector.tensor_tensor(out=var, in0=nmu, in1=nmu, op=OP.mult)
            nc.vector.tensor_tensor(out=var, in0=ex2, in1=var, op=OP.subtract)
            nc.scalar.activation(
                out=rstd, in_=var, func=AF.Sqrt, bias=eps_col, scale=1.0
            )
            nc.vector.reciprocal(out=rstd, in_=rstd)
            # apply (x - mu) * rstd (add on gpsimd, mult+round on DVE)
            xc = mid.tile([128, NCH * 119], f32, tag="xc")
            xcg = xc[:, : NCH * T].rearrange("p (g j) -> p g j", j=J)
            nc.gpsimd.tensor_tensor(
                out=xcg,
                in0=xTg,
                in1=nmu[:, :, None].to_broadcast((128, G, J)),
                op=OP.add,
            )
            xgT = mid.tile([128, NCH * 119], f32, tag="xgT")
            xgTg = xgT[:, : NCH * T].rearrange("p (g j) -> p g j", j=J)
            nc.vector.tensor_tensor(
                out=r32(xgTg),
                in0=xcg,
                in1=rstd[:, :, None].to_broadcast((128, G, J)),
                op=OP.mult,
            )
            return dict(S=S, T=T, t0=t0, x_t=x_t, ba=ba, xgT=xgT)

        def heavy_a(fr):
            """conv1 + agg1 + g1 transpose."""
            T = fr["T"]
            pg1 = conv_gcn(
                fr["xgT"][:, : NCH * T].rearrange("p (c t) -> p c t", t=T),
                T, W1T, B0s1, Cs1, s1_col, gJ_col, fr["ba"],
            )
            g1_sb = scr_pool.tile([128, C], f32, tag="g1")
            nc.scalar.copy(out=g1_sb[:T], in_=pg1[:T])
            fr["g1T"] = transpose_tokmaj(g1_sb, T, "g1T", round_out=True)

        def heavy_b(fr, gs):
            """conv2+agg2, residual, LN2, xn transpose."""
            T = fr["T"]
            x_t = fr["x_t"]
            pg2 = conv_gcn(fr["g1T"], T, W2T, B0s2, Cs2, s2_col, None, fr["ba"])
            # x2 = x + g2 (DVE add straight out of PSUM, f32r-rounded)
            x2_sb = x2pool.tile([128, C], f32, tag="x2")
            nc.vector.tensor_tensor(
                out=r32(x2_sb[:T]), in0=pg2[:T], in1=x_t[:T], op=OP.add
            )

            # ---- LN2 (token-major; gamma2/beta2 folded into w1/b1) ----
            st2 = small.tile([128, 8], f32, tag="st2")
            nc.vector.tensor_reduce(
                out=st2[:T, 0:1], in_=x2_sb[:T], axis=AX.X, op=OP.add, negate=True
            )
            scr2 = scr_pool.tile([128, C], f32, tag="scr2")
            nc.scalar.activation(
                out=scr2[:T],
                in_=x2_sb[:T],
                func=AF.Square,
                accum_out=st2[:T, 1:2],
            )
            nc.vector.tensor_scalar(
                out=st2[:T, 2:3], in0=st2[:T, 0:1], scalar1=invC,
                scalar2=None, op0=OP.mult,
            )
            nc.vector.tensor_scalar(
                out=st2[:T, 3:4], in0=st2[:T, 1:2], scalar1=invC,
                scalar2=None, op0=OP.mult,
            )
            nc.vector.tensor_tensor(
                out=st2[:T, 4:5], in0=st2[:T, 2:3], in1=st2[:T, 2:3], op=OP.mult
            )
            nc.vector.tensor_tensor(
                out=st2[:T, 4:5], in0=st2[:T, 3:4], in1=st2[:T, 4:5],
                op=OP.subtract,
            )
            nc.scalar.activation(
                out=st2[:T, 5:6], in_=st2[:T, 4:5], func=AF.Sqrt,
                bias=eps_col[:T], scale=1.0,
            )
            nc.vector.reciprocal(out=st2[:T, 5:6], in_=st2[:T, 5:6])
            xn_sb = scr_pool.tile([128, C], f32, tag="xn")
            nc.vector.tensor_scalar(
                out=xn_sb[:T],
                in0=x2_sb[:T],
                scalar1=st2[:T, 2:3],
                scalar2=st2[:T, 5:6],
                op0=OP.add,
                op1=OP.mult,
            )

            # ---- transpose xn into the group buffer ----
            pt = pt_pool.tile([128, 512], f32, tag="pt")
            for c in range(NCH):
                tmm(
                    pt[:, c * T : (c + 1) * T],
                    xn_sb[:T, c * 128 : (c + 1) * 128],
                    ident[:T, :T],
                    start=(c == 0),
                    stop=(c == NCH - 1),
                )
            off = gs["off"]
            nc.vector.tensor_copy(
                out=r32(gs["xnT_gr"][:, :, off : off + T]),
                in_=pt[:, : NCH * T].rearrange("p (c t) -> p c t", t=T),
            )
            gs["x2"].append(x2_sb)
            gs["offs"].append(off)
            gs["Ts"].append(T)
            gs["t0s"].append(fr["t0"])
            gs["off"] = off + T

        def umlp_ab(gs):
            W = gs["off"]
            xnT_gr = gs["xnT_gr"]
            h1_sb = grp_pool.tile([128, H // 128, GRP * 119], f32, tag="h1")
            for m in range(H // 128):
                pu = pu_pool.tile([128, GRP * 119], f32, tag="pu")
                for kc in range(NCH):
                    mm(
                        pu[:, :W],
                        w1r[:, kc, m * 128 : (m + 1) * 128],
                        xnT_gr[:, kc, :W],
                        start=(kc == 0),
                        stop=(kc == NCH - 1),
                    )
                nc.scalar.activation(
                    out=r32(h1_sb[:, m, :W]), in_=pu[:, :W], func=AF.Gelu,
                    bias=b1_col[:, m : m + 1], scale=1.0,
                )
            h2_sb = grp_pool.tile([128, H // 128, GRP * 119], f32, tag="h2")
            for m in range(H // 128):
                pu = pu_pool.tile([128, GRP * 119], f32, tag="pu")
                for kc in range(H // 128):
                    mm(
                        pu[:, :W],
                        w2_sb[:, kc, m * 128 : (m + 1) * 128],
                        h1_sb[:, kc, :W],
                        start=(kc == 0),
                        stop=(kc == H // 128 - 1),
                    )
                nc.scalar.activation(
                    out=r32(h2_sb[:, m, :W]), in_=pu[:, :W], func=AF.Gelu,
                    bias=b2_col[:, m : m + 1], scale=1.0,
                )
            gs["h1"] = h1_sb
            gs["h2"] = h2_sb

        def umlp_c(gs):
            W = gs["off"]
            h1_sb = gs["h1"]
            h2_sb = gs["h2"]
            h3_sb = grp_pool.tile([128, NCH, GRP * 119], f32, tag="h3")
            for m in range(NCH):
                pu = pu_pool.tile([128, GRP * 119], f32, tag="pu")
                for kc in range(H // 128):
                    # (h2g + h1) @ w3: residual folded as a second accumulate
                    mm(
                        pu[:, :W],
                        w3_sb[:, kc, m * 128 : (m + 1) * 128],
                        h2_sb[:, kc, :W],
                        start=(kc == 0),
                        stop=False,
                    )
                    mm(
                        pu[:, :W],
                        w3_sb[:, kc, m * 128 : (m + 1) * 128],
                        h1_sb[:, kc, :W],
                        start=False,
                        stop=(kc == H // 128 - 1),
                    )
                nc.scalar.activation(
                    out=h3_sb[:, m, :W], in_=pu[:, :W], func=AF.Gelu,
                    bias=b3_col[:, m : m + 1], scale=1.0,
                )

            # ---- per tile: transpose back to token-major, + x2, store ----
            for i in range(len(gs["Ts"])):
                T = gs["Ts"][i]
                t0 = gs["t0s"][i]
                po = pg_pool.tile([128, 512], f32, tag="pg")
                for c in range(NCH):
                    tmm(
                        po[:T, c * 128 : (c + 1) * 128],
                        h3_sb[:, c, gs["offs"][i] : gs["offs"][i] + T],
                        ident,
                        start=(c == 0),
                        stop=False,
                    )
                mm(po[:T], identr[:T, :T], gs["x2"][i][:T], start=False, stop=True)
                out_sb = opool.tile([128, C], f32, tag="out")
                nc.scalar.copy(out=out_sb[:T], in_=po[:T])
                nc.sync.dma_start(out=yf[t0 : t0 + T], in_=out_sb[:T])

        gs = None
        gs_done = None  # group whose uMLP is pending, interleaved with next tiles
        fr_next = front(0)
        for _t in range(ntiles):
            fr = fr_next
            if _t + 1 < ntiles:
                fr_next = front(_t + 1)
            if _t % GRP == 0:
                gs = dict(
                    xnT_gr=grp2_pool.tile(
                        [128, NCH, GRP * 119], f32, tag="xnT", name="xnT_gr"
                    ),
                    x2=[], offs=[], Ts=[], t0s=[], off=0,
                )
            heavy_a(fr)
            if gs_done is not None:
                umlp_ab(gs_done)
            heavy_b(fr, gs)
            if gs_done is not None:
                umlp_c(gs_done)
                gs_done = None
            if _t % GRP == GRP - 1 or _t == ntiles - 1:
                gs_done = gs
        umlp_ab(gs_done)
        umlp_c(gs_done)

    nc.compile()
    return nc


def _get(bs: int):
    if bs not in _cache:
        _cache[bs] = _build(bs)
    return _cache[bs]


def kernel(**inputs) -> np.ndarray:
    from concourse.bass_utils import run_bass_kernel_spmd

    x = np.ascontiguousarray(np.asarray(inputs["x"], dtype=np.float32))
    adj = np.ascontiguousarray(np.asarray(inputs["adj"], dtype=np.float32))
    rest = {
        k: np.ascontiguousarray(np.asarray(v, dtype=np.float32))
        for k, v in inputs.items()
        if k not in ("x", "adj")
    }
    bs = x.shape[0] // NCORES
    nc = _get(bs)
    in_maps = []
    for c in range(NCORES):
        m = {"x": x[c * bs : (c + 1) * bs], "adj": adj[c * bs : (c + 1) * bs]}
        m.update(rest)
        in_maps.append(m)
    res = run_bass_kernel_spmd(nc, in_maps, core_ids=list(range(NCORES)))
    return np.concatenate([res.results[c]["y"] for c in range(NCORES)], axis=0)
